# revision 1
# baseline (speedup 1.0000x reference)
"""Trainium2 Bass kernel for the dynamic-kernel ECA module.

Computation per sample:
  gap  = mean(x, axis=l)                       (c,)
  h    = gelu(gap @ w1.T + b1)                 (hidden,)
  th   = tanh(h @ w2.T + b2); delta = 2*th     scalar
  k    = (5 + clip(round(delta), -3, 3)) | 1   in {3,5,7} (delta in (-2,2))
  w    = box filter of width k in 9-tap window, 1/k weights
  y    = conv1d(gap, w) along c (zero pad 4)   (c,)
  s    = sigmoid(y)
  out  = x * s[:, None]

Sharding: pure data parallel, batch 16 -> 8 cores x 2 samples.

Memory strategy (per core): x moves through HBM in bf16 (the 2e-2
rel-err budget gives ~10x margin over bf16's ~2e-3 quantization; the
kernel is purely HBM-bandwidth-bound at ~420 GB/s/core).  The 16 MiB
bf16 shard fits in SBUF entirely, so x is read exactly once and out
written exactly once (32 MiB/core of traffic ~= 80 us of DMA).

Queue discipline: the SP HWDGE queue carries ONLY the 16 x-tile loads
(an in-order sequencer; any gate-dependent wait interleaved there
head-of-line-blocks the remaining loads).  All constants ship as a
single packed transfer on the ScalarE HWDGE queue, which also carries
the 8 stores.  The first store is gated on the third-to-last load
(completion receipts lag data by a few us under full DMA load), giving
a pure read phase then a pure write phase.

The gate is built to minimize serial cross-engine hops, which at ~1 us
each (sem latency + engine-stream contention) dominate its latency:
PE runs the MLP matmuls AND all nine candidate band-conv matmuls
(k in {3,5,7} x in-chunk/hi-wrap/lo-wrap, host-precomputed constant
lhsT) back to back; ScalarE turns each candidate PSUM into
tanh(y_k/2); the scalar th is broadcast across partitions by a
ones-row matmul; VectorE then computes the three k-selection flags and
blends the three candidate results (sigmoid(y) = 0.5 + 0.5*tanh(y/2),
and sum(flags) = 1, so the affine folds into one op).  Nothing
data-dependent ever touches the matmul weights.

Load reductions (l-sums) are split per 1 MiB half-load and spread over
VectorE/ScalarE by explicit deadline order: sample-1's tail reductions
gate gate1 -> s1 scales -> s1 stores, so they get dedicated slots;
VectorE-assigned ones use a bf16 2x-mode fold-add before a half-size
1x reduce.  1/L is folded into w1 and the band weights on the host.
"""

import os
from contextlib import ExitStack

import numpy as np
import ml_dtypes

import concourse.bacc as bacc
import concourse.mybir as mybir
import concourse.tile as tile
from concourse.tile_rust import add_dep_helper
from concourse.bass_utils import run_bass_kernel_spmd

F32 = mybir.dt.float32
BF16 = mybir.dt.bfloat16
ALU = mybir.AluOpType
ACTF = mybir.ActivationFunctionType
AX_X = mybir.AxisListType.X

B, C, L = 16, 512, 8192
HID = 64
N_CORES = 8
BS = B // N_CORES            # samples per core = 2
CP = C // 128                # channel chunks = 4
NH = 2                       # load halves per tile
HL = L // NH                 # 4096 elements = 1 MiB bf16 per half-load

# packed const blob layout (f32 columns)
W1T_OFF = 0                  # [128, CP*HID] = 256 cols
WB_OFF = 256                 # [128, 3*3*128] = 1152 cols (m-major, k-minor)
B1_OFF = 1408                # [64, 1]
W2T_OFF = 1409               # [64, 1]
CST_COLS = 1410

S0_RED = ['v', 'a', 'v', 'a', 'v', 'a', 'v', 'a']


def _inst(x):
    return getattr(x, "ins", x)


def _build(b2_val):
    nc = bacc.Bacc("TRN2", target_bir_lowering=False, debug=False,
                   num_devices=N_CORES)

    x_d = nc.dram_tensor("x", [BS, C, L], BF16, kind="ExternalInput").ap()
    cst_d = nc.dram_tensor("cst", [128, CST_COLS], F32,
                           kind="ExternalInput").ap()
    o_d = nc.dram_tensor("out", [BS, C, L], BF16, kind="ExternalOutput").ap()

    with ExitStack() as ctx:
        tc = ctx.enter_context(tile.TileContext(nc))
        cache = ctx.enter_context(tc.tile_pool(name="cache", bufs=1))
        small = ctx.enter_context(tc.tile_pool(name="small", bufs=1))
        psum = ctx.enter_context(tc.tile_pool(name="psum", bufs=1, space="PSUM"))

        xt = {}
        partials = {}
        ge = {}
        loads = []

        def wb(m, kidx):
            o = WB_OFF + (m * 3 + kidx) * 128
            return cst[:, o:o + 128]

        def reduce_half(s, ci, h, eng):
            t = xt[(s, ci)][:, h * HL:(h + 1) * HL]
            dst = partials[s][:, ci, h:h + 1]
            if eng == 'v':
                nc.vector.reduce_sum(out=dst, in_=t, axis=AX_X)
            elif eng == 'vf':
                # bf16 fold-add at 2x mode, then a half-size 1x reduce
                f = small.tile([128, HL // 2], BF16, tag="fold")
                nc.vector.tensor_add(f[:], t[:, 0:HL // 2], t[:, HL // 2:HL])
                nc.vector.reduce_sum(out=dst, in_=f[:], axis=AX_X)
            else:
                nc.scalar.activation(t, t, ACTF.Copy, accum_out=dst)

        # ---- init + packed consts (single ScalarE-queue transfer) --------
        ge[0] = small.tile([128, CP + 2], F32, tag="ge0", name="ge0")
        ge[1] = small.tile([128, CP + 2], F32, tag="ge1", name="ge1")
        ones = small.tile([1, 128], F32, tag="ones")
        b2t = small.tile([1, 1], F32, tag="b2t")
        nc.vector.memset(ge[0][:], 0.0)
        nc.vector.memset(ge[1][:], 0.0)
        nc.vector.memset(ones[:], 1.0)
        nc.vector.memset(b2t[:], float(b2_val))
        cst = small.tile([128, CST_COLS], F32, tag="cst")
        nc.scalar.dma_start(out=cst[:], in_=cst_d[:])
        b1 = cst[0:HID, B1_OFF:B1_OFF + 1]
        w2t = cst[0:HID, W2T_OFF:W2T_OFF + 1]

        # ---- pass 1: x loads on the SP HWDGE queue + l-sum reductions ----
        def load_sample(s, red):
            partials[s] = small.tile([128, CP, NH], F32,
                                     tag=f"partials{s}", name=f"partials{s}")
            for ci in range(CP):
                t = cache.tile([128, L], BF16, tag=f"x{s}{ci}",
                               name=f"x{s}{ci}")
                xt[(s, ci)] = t
                for h in range(NH):
                    ld = nc.sync.dma_start(
                        out=t[:, h * HL:(h + 1) * HL],
                        in_=x_d[s, ci * 128:(ci + 1) * 128,
                                h * HL:(h + 1) * HL])
                    loads.append(ld)
                    if red is not None:
                        reduce_half(s, ci, h, red[ci * NH + h])

        def merge_gmean(s):
            nc.vector.reduce_sum(out=ge[s][:, 1:1 + CP], in_=partials[s][:],
                                 axis=AX_X)

        load_sample(0, S0_RED)
        merge_gmean(0)
        load_sample(1, None)

        # ---- per-sample gate (hop-minimized) -----------------------------
        def gate_sample(s):
            # PE: MLP layer 1, then all nine constant band-conv matmuls
            hp = psum.tile([HID, 1], F32, tag="hp")
            for i in range(CP):
                nc.tensor.matmul(hp[:], lhsT=cst[:, i * HID:(i + 1) * HID],
                                 rhs=ge[s][:, 1 + i:2 + i],
                                 start=(i == 0), stop=(i == CP - 1))
            yk = []
            for kidx in range(3):
                yp = psum.tile([128, CP], F32, tag=f"y{kidx}")
                nc.tensor.matmul(yp[:], lhsT=wb(0, kidx),
                                 rhs=ge[s][:, 1:1 + CP], start=True,
                                 stop=False)
                nc.tensor.matmul(yp[:], lhsT=wb(1, kidx),
                                 rhs=ge[s][:, 2:2 + CP], start=False,
                                 stop=False)
                nc.tensor.matmul(yp[:], lhsT=wb(2, kidx),
                                 rhs=ge[s][:, 0:CP], start=False, stop=True)
                yk.append(yp)

            h = small.tile([HID, 1], F32, tag="h")
            nc.scalar.activation(h[:], hp[:], ACTF.Gelu, bias=b1, scale=1.0)
            dp = psum.tile([1, 1], F32, tag="dp")
            nc.tensor.matmul(dp[:], lhsT=h[:], rhs=w2t, start=True, stop=True)

            # th = tanh(dp + b2) with the bias fused into the activation
            th = small.tile([1, 1], F32, tag="th")
            nc.scalar.activation(th[:], dp[:], ACTF.Tanh, bias=b2t[:],
                                 scale=1.0)
            # candidate sigmoid halves: tk_k = tanh(y_k / 2)
            tk = []
            for kidx in range(3):
                tt = small.tile([128, CP], F32, tag=f"tk{kidx}")
                nc.scalar.activation(tt[:], yk[kidx][:], ACTF.Tanh, scale=0.5)
                tk.append(tt)

            # broadcast th across partitions, then flags + blend on VectorE
            thp = psum.tile([128, 1], F32, tag="thp")
            nc.tensor.matmul(thp[:], lhsT=ones[:], rhs=th[:], start=True,
                             stop=True)
            fb = small.tile([128, 3], F32, tag="fb")
            nc.vector.tensor_scalar(out=fb[:, 0:1], in0=thp[:], scalar1=0.25,
                                    scalar2=None, op0=ALU.is_ge)
            nc.vector.tensor_scalar(out=fb[:, 1:2], in0=thp[:], scalar1=-0.75,
                                    scalar2=None, op0=ALU.is_lt)
            nc.vector.tensor_add(fb[:, 2:3], fb[:, 0:1], fb[:, 1:2])
            nc.vector.tensor_scalar(out=fb[:, 2:3], in0=fb[:, 2:3],
                                    scalar1=-1.0, scalar2=1.0, op0=ALU.mult,
                                    op1=ALU.add)
            # sg = 0.5 + 0.5*(bb*tk3 + u*tk5 + a*tk7)   (flags sum to 1)
            bl = small.tile([128, CP], F32, tag="bl")
            t2 = small.tile([128, CP], F32, tag="t2")
            nc.vector.tensor_scalar(out=bl[:], in0=tk[0][:],
                                    scalar1=fb[:, 1:2], scalar2=None,
                                    op0=ALU.mult)
            nc.vector.tensor_scalar(out=t2[:], in0=tk[1][:],
                                    scalar1=fb[:, 2:3], scalar2=None,
                                    op0=ALU.mult)
            nc.vector.tensor_add(bl[:], bl[:], t2[:])
            nc.vector.tensor_scalar(out=t2[:], in0=tk[2][:],
                                    scalar1=fb[:, 0:1], scalar2=None,
                                    op0=ALU.mult)
            nc.vector.tensor_add(bl[:], bl[:], t2[:])
            sg = small.tile([128, CP], F32, tag=f"sg{s}")
            nc.vector.tensor_scalar(out=sg[:], in0=bl[:], scalar1=0.5,
                                    scalar2=0.5, op0=ALU.mult, op1=ALU.add)
            return sg

        sg = {}
        sg[0] = gate_sample(0)

        def scale_tile(s, ci):
            t = xt[(s, ci)]
            nc.vector.tensor_scalar_mul(t[:], t[:], sg[s][:, ci:ci + 1])

        def store_tile(s, ci):
            return nc.scalar.dma_start(
                out=o_d[s, ci * 128:(ci + 1) * 128, :],
                in_=xt[(s, ci)][:])

        # ---- s1 reductions / s0 scales / s0 stores in deadline order -----
        # ScalarE: reds 0,1,3,5,6,7 (+ store triggers slotted between);
        # VectorE: folds for reds 2,4 between the s0 scale muls.
        reduce_half(1, 0, 0, 'a')
        reduce_half(1, 0, 1, 'a')
        scale_tile(0, 0)
        reduce_half(1, 1, 0, 'vf')
        reduce_half(1, 1, 1, 'a')
        scale_tile(0, 1)
        reduce_half(1, 2, 0, 'vf')
        reduce_half(1, 2, 1, 'a')
        st0 = store_tile(0, 0)
        add_dep_helper(_inst(st0), _inst(loads[-3]), sync=True,
                       reason="stores after load phase")
        reduce_half(1, 3, 0, 'a')
        store_tile(0, 1)
        reduce_half(1, 3, 1, 'a')
        merge_gmean(1)
        scale_tile(0, 2)
        store_tile(0, 2)
        scale_tile(0, 3)
        store_tile(0, 3)

        sg[1] = gate_sample(1)
        for ci in range(CP):
            scale_tile(1, ci)
        for ci in range(CP):
            store_tile(1, ci)

    nc.compile()
    return nc


_COMPILED = {}


def _get_compiled(b2_val):
    key = float(b2_val)
    if key not in _COMPILED:
        _COMPILED[key] = _build(key)
    return _COMPILED[key]


def _make_consts(w1, b1, w2, b2):
    w1 = np.asarray(w1, np.float32)
    b1 = np.asarray(b1, np.float32)
    w2 = np.asarray(w2, np.float32)

    S17 = np.zeros((128, 17, 128), np.float32)
    p = np.arange(128)
    for j in range(9):
        d = j - 4
        m = (p + d >= 0) & (p + d < 128)
        S17[p[m] + d, j, p[m]] = 1.0
    for d in range(1, 5):
        m = p + d - 128 >= 0
        S17[p[m] + d - 128, 8 + d, p[m]] = 1.0
    for d in range(-4, 0):
        m = p + d + 128 < 128
        S17[p[m] + d + 128, 17 + d, p[m]] = 1.0

    cst = np.zeros((128, CST_COLS), np.float32)
    # w1t: [CP, 128, HID] flattened as CP blocks of HID columns, 1/L folded
    w1t = (w1.T / np.float32(L)).reshape(CP, 128, HID)
    for i in range(CP):
        cst[:, W1T_OFF + i * HID:W1T_OFF + (i + 1) * HID] = w1t[i]
    j9 = np.arange(9)
    for kidx, k in enumerate((3, 5, 7)):
        w = ((np.abs(j9 - 4) <= (k - 1) // 2).astype(np.float32)
             / np.float32(k) / np.float32(L))
        bands = [sum(w[j] * S17[:, j, :] for j in range(9)),
                 sum(w[d + 4] * S17[:, 8 + d, :] for d in range(1, 5)),
                 sum(w[d + 4] * S17[:, 17 + d, :] for d in range(-4, 0))]
        for m in range(3):
            o = WB_OFF + (m * 3 + kidx) * 128
            cst[:, o:o + 128] = bands[m]
    cst[0:HID, B1_OFF] = b1
    cst[0:HID, W2T_OFF] = w2.reshape(HID)
    return {"cst": np.ascontiguousarray(cst)}


def kernel(x, w1, b1, w2, b2):
    x = np.asarray(x, np.float32)
    assert x.shape == (B, C, L), x.shape
    nc = _get_compiled(np.float32(np.asarray(b2).reshape(-1)[0]))
    consts = _make_consts(w1, b1, w2, b2)
    xb = np.ascontiguousarray(x).astype(ml_dtypes.bfloat16)
    in_maps = []
    for i in range(N_CORES):
        m = {"x": np.ascontiguousarray(xb[i * BS:(i + 1) * BS])}
        m.update(consts)
        in_maps.append(m)
    res = run_bass_kernel_spmd(nc, in_maps, list(range(N_CORES)),
                               trace=bool(int(os.environ.get("K_TRACE", "0"))))
    out = np.concatenate(
        [np.asarray(res.results[i]["out"]).astype(np.float32)
         for i in range(N_CORES)], axis=0)
    if res.exec_time_ns is not None:
        kernel.last_exec_time_ns = res.exec_time_ns
        kernel.last_mean_exec_time_ns = res.mean_exec_time_ns
    kernel.last_results = res
    return out



# revision 13
# speedup vs baseline: 1.0806x; 1.0806x over previous
"""Trainium2 Bass kernel for the dynamic-kernel ECA module.

Computation per sample:
  gap  = mean(x, axis=l)                       (c,)
  h    = gelu(gap @ w1.T + b1)                 (hidden,)
  th   = tanh(h @ w2.T + b2); delta = 2*th     scalar
  k    = (5 + clip(round(delta), -3, 3)) | 1   in {3,5,7} (delta in (-2,2))
  w    = box filter of width k in 9-tap window, 1/k weights
  y    = conv1d(gap, w) along c (zero pad 4)   (c,)
  s    = sigmoid(y)
  out  = x * s[:, None]

Sharding: pure data parallel, batch 16 -> 8 cores x 2 samples.

The kernel is HBM-bandwidth-bound: 16 MiB of bf16 x in + 16 MiB out
per core at the ~427 GB/s/core SDMA fabric rate = ~78.6 us of pure
data movement, plus ~8 us fixed NEFF preamble.  Everything else is
scheduled to hide under the DMA stream:

  * ALL x traffic (18 load pieces then 9 stores) rides ONE HWDGE ring
    (SP/sync).  FIFO order on the ring makes the load->store transition
    seamless: store descriptors are enqueued long before the loads
    drain, so the ring never idles between phases.  The tiny constant
    blob rides the other (scalar/ACT) ring and overlaps the first
    loads.
  * l-sum reductions are split per half-tile: the even halves are
    fused fold+reduce ops on VectorE (tensor_tensor_reduce: one pass
    over 2048 bf16 pairs), the odd halves are ScalarE activation-
    accumulate.  The last half of each sample is split into two
    512 KiB quarter-loads so the final partial reduce trails the last
    receipt by ~1 us instead of ~4.
  * The per-sample gate (MLP -> k flags -> 3-candidate band-conv
    blend) runs in bf16 on the PE with host-precomputed constant
    lhsT matrices; ScalarE turns candidate PSUMs into tanh(y/2);
    VectorE blends (sigmoid(y) = 0.5 + 0.5*tanh(y/2), flags sum to 1).
    Dummy Gelu/Tanh activations at program start hoist the ACT table
    loads off the gate critical path.
  * VectorE's stream is deadline-ordered: sample-1 fold+reduces are
    interleaved between sample-0 scale muls so every store's scale is
    done several us before the ring reaches that store.
"""

import os
from contextlib import ExitStack

import numpy as np
import ml_dtypes

import concourse.bacc as bacc
import concourse.mybir as mybir
import concourse.tile as tile
from concourse.bass_utils import run_bass_kernel_spmd

F32 = mybir.dt.float32
BF16 = mybir.dt.bfloat16
ALU = mybir.AluOpType
ACTF = mybir.ActivationFunctionType
AX_X = mybir.AxisListType.X

GELU = ACTF.Gelu             # simtest overrides (CoreSim lacks Gelu)

# bisection switches (hardcoded defaults; env only used while debugging)
K_TTR = int(os.environ.get("K_TTR", "2"))  # 0=fold+reduce 1=ttr 2=stt+accum
K_ST_SYNC = bool(int(os.environ.get("K_ST_SYNC", "1")))  # stores on sync ring
K_STT = bool(int(os.environ.get("K_STT", "1")))       # scalar_tensor_tensor
K_DMY = bool(int(os.environ.get("K_DMY", "1")))       # ACT table prefetch

B, C, L = 16, 512, 8192
HID = 64
N_CORES = 8
BS = B // N_CORES            # samples per core = 2
CP = C // 128                # channel chunks = 4
HL = L // 2                  # 4096 (1 MiB bf16 half-tile)
QL = HL // 2                 # 2048 (512 KiB quarter)

# packed bf16 const blob layout (columns)
W1T_OFF = 0                  # [128, CP*HID] = 256 cols
WB_OFF = 256                 # [128, 3*3*128] = 1152 cols (m-major, k-minor)
CB_COLS = 1408


def _build(b2_val):
    nc = bacc.Bacc("TRN2", target_bir_lowering=False, debug=False,
                   num_devices=N_CORES)

    x_d = nc.dram_tensor("x", [BS, C, L], BF16, kind="ExternalInput").ap()
    cstb_d = nc.dram_tensor("cstb", [128, CB_COLS], BF16,
                            kind="ExternalInput").ap()
    cstf_d = nc.dram_tensor("cstf", [HID, 2], F32, kind="ExternalInput").ap()
    o_d = nc.dram_tensor("out", [BS, C, L], BF16, kind="ExternalOutput").ap()

    with ExitStack() as ctx:
        tc = ctx.enter_context(tile.TileContext(nc))
        cache = ctx.enter_context(tc.tile_pool(name="cache", bufs=1))
        small = ctx.enter_context(tc.tile_pool(name="small", bufs=1))
        psum = ctx.enter_context(tc.tile_pool(name="psum", bufs=1, space="PSUM"))

        xt = {}

        def wb(m, kidx):
            o = WB_OFF + (m * 3 + kidx) * 128
            return cstb[:, o:o + 128]

        # ---- init: memsets (V), ACT-table prefetch (A), const loads ------
        ge = {s: small.tile([128, CP + 2], BF16, tag=f"ge{s}", name=f"ge{s}")
              for s in range(BS)}
        partials = {s: small.tile([128, CP, 3], F32, tag=f"par{s}",
                                  name=f"par{s}") for s in range(BS)}
        ones = small.tile([1, 128], F32, tag="ones")
        b2t = small.tile([1, 1], F32, tag="b2t")
        dmy = small.tile([1, 3], F32, tag="dmy")
        for s in range(BS):
            nc.vector.memset(ge[s][:], 0.0)
            nc.vector.memset(partials[s][:], 0.0)
        nc.vector.memset(ones[:], 1.0)
        nc.vector.memset(b2t[:], float(b2_val))
        nc.vector.memset(dmy[:], 0.0)
        if K_DMY:
            # hoist ACT table loads off the gate critical path
            nc.scalar.activation(dmy[:, 1:2], dmy[:, 0:1], GELU)
            nc.scalar.activation(dmy[:, 2:3], dmy[:, 0:1], ACTF.Tanh)

        cstb = small.tile([128, CB_COLS], BF16, tag="cstb")
        cstf = small.tile([HID, 2], F32, tag="cstf")
        nc.scalar.dma_start(out=cstb[:], in_=cstb_d[:])
        nc.scalar.dma_start(out=cstf[:], in_=cstf_d[:])
        b1 = cstf[0:HID, 0:1]
        w2t = cstf[0:HID, 1:2]

        # ---- all 18 x-load triggers up front on the sync (SP) ring -------
        # per (s, ci): piece 0 = [0:HL], piece 1 = [HL:HL+?]; the last tile
        # of each sample splits piece 1 into two quarters.
        for s in range(BS):
            for ci in range(CP):
                t = cache.tile([128, L], BF16, tag=f"x{s}{ci}",
                               name=f"x{s}{ci}")
                xt[(s, ci)] = t
                cuts = ([0, HL, L] if ci < CP - 1 else [0, HL, HL + QL, L])
                for a, b in zip(cuts[:-1], cuts[1:]):
                    nc.sync.dma_start(out=t[:, a:b],
                                      in_=x_d[s, ci * 128:(ci + 1) * 128, a:b])

        scratch = small.tile([128, QL], BF16, tag="ttr_scratch")

        def red_v(s, ci):
            # fused fold+reduce of the even half on VectorE
            t = xt[(s, ci)]
            if K_TTR == 1:
                nc.vector.tensor_tensor_reduce(
                    out=scratch[:], in0=t[:, 0:QL], in1=t[:, QL:HL],
                    scale=1.0, scalar=0.0, op0=ALU.add, op1=ALU.add,
                    accum_out=partials[s][:, ci, 0:1])
            elif K_TTR == 2:
                nc.vector.scalar_tensor_tensor(
                    out=scratch[:], in0=t[:, 0:QL], scalar=1.0,
                    in1=t[:, QL:HL], op0=ALU.mult, op1=ALU.add,
                    accum_out=partials[s][:, ci, 0:1])
            else:
                nc.vector.tensor_add(scratch[:], t[:, 0:QL], t[:, QL:HL])
                nc.vector.reduce_sum(out=partials[s][:, ci, 0:1],
                                     in_=scratch[:], axis=AX_X)

        def red_a(s, ci, a, b, slot):
            t = xt[(s, ci)][:, a:b]
            nc.scalar.activation(t, t, ACTF.Copy,
                                 accum_out=partials[s][:, ci, slot:slot + 1])

        def red_sample(s):
            for ci in range(CP):
                red_v(s, ci)
                if ci < CP - 1:
                    red_a(s, ci, HL, L, 1)
                else:
                    red_a(s, ci, HL, HL + QL, 1)
                    red_a(s, ci, HL + QL, L, 2)

        def merge(s):
            geF = small.tile([128, CP], F32, tag=f"geF{s}")
            nc.vector.reduce_sum(out=geF[:], in_=partials[s][:], axis=AX_X)
            nc.vector.tensor_scalar(out=ge[s][:, 1:1 + CP], in0=geF[:],
                                    scalar1=1.0, scalar2=None, op0=ALU.mult)

        # ---- per-sample gate: PE matmuls + A activations -----------------
        def gate_mm(s):
            hp = psum.tile([HID, 1], F32, tag="hp")
            for i in range(CP):
                nc.tensor.matmul(hp[:],
                                 lhsT=cstb[:, W1T_OFF + i * HID:
                                           W1T_OFF + (i + 1) * HID],
                                 rhs=ge[s][:, 1 + i:2 + i],
                                 start=(i == 0), stop=(i == CP - 1))
            yk = []
            for kidx in range(3):
                yp = psum.tile([128, CP], F32, tag=f"y{kidx}")
                nc.tensor.matmul(yp[:], lhsT=wb(0, kidx),
                                 rhs=ge[s][:, 1:1 + CP], start=True,
                                 stop=False)
                nc.tensor.matmul(yp[:], lhsT=wb(1, kidx),
                                 rhs=ge[s][:, 2:2 + CP], start=False,
                                 stop=False)
                nc.tensor.matmul(yp[:], lhsT=wb(2, kidx),
                                 rhs=ge[s][:, 0:CP], start=False, stop=True)
                yk.append(yp)

            h = small.tile([HID, 1], F32, tag="h")
            nc.scalar.activation(h[:], hp[:], GELU, bias=b1, scale=1.0)
            dp = psum.tile([1, 1], F32, tag="dp")
            nc.tensor.matmul(dp[:], lhsT=h[:], rhs=w2t, start=True, stop=True)
            th = small.tile([1, 1], F32, tag="th")
            nc.scalar.activation(th[:], dp[:], ACTF.Tanh, bias=b2t[:],
                                 scale=1.0)
            # candidate sigmoid halves tk_k = tanh(y_k / 2); th broadcast
            tk = []
            for kidx in range(3):
                tt = small.tile([128, CP], F32, tag=f"tk{kidx}")
                nc.scalar.activation(tt[:], yk[kidx][:], ACTF.Tanh, scale=0.5)
                tk.append(tt)
            thp = psum.tile([128, 1], F32, tag="thp")
            nc.tensor.matmul(thp[:], lhsT=ones[:], rhs=th[:], start=True,
                             stop=True)
            return tk, thp

        def gate_blend(s, tk, thp):
            # flags: a = [th >= 0.25] (k=7), bb = [th < -0.75] (k=3)
            fb = small.tile([128, 2], F32, tag="fb")
            nc.vector.tensor_scalar(out=fb[:, 0:1], in0=thp[:], scalar1=0.25,
                                    scalar2=None, op0=ALU.is_ge)
            nc.vector.tensor_scalar(out=fb[:, 1:2], in0=thp[:], scalar1=-0.75,
                                    scalar2=None, op0=ALU.is_lt)
            # bl = tk5 + a*(tk7-tk5) + bb*(tk3-tk5);  sg = 0.5 + 0.5*bl
            u = small.tile([128, CP], F32, tag="u")
            bl = small.tile([128, CP], F32, tag="bl")
            nc.vector.tensor_sub(u[:], tk[2][:], tk[1][:])
            if K_STT:
                nc.vector.scalar_tensor_tensor(out=bl[:], in0=u[:],
                                               scalar=fb[:, 0:1],
                                               in1=tk[1][:],
                                               op0=ALU.mult, op1=ALU.add)
            else:
                nc.vector.tensor_scalar(out=u[:], in0=u[:],
                                        scalar1=fb[:, 0:1], scalar2=None,
                                        op0=ALU.mult)
                nc.vector.tensor_add(bl[:], u[:], tk[1][:])
            nc.vector.tensor_sub(u[:], tk[0][:], tk[1][:])
            if K_STT:
                nc.vector.scalar_tensor_tensor(out=bl[:], in0=u[:],
                                               scalar=fb[:, 1:2], in1=bl[:],
                                               op0=ALU.mult, op1=ALU.add)
            else:
                nc.vector.tensor_scalar(out=u[:], in0=u[:],
                                        scalar1=fb[:, 1:2], scalar2=None,
                                        op0=ALU.mult)
                nc.vector.tensor_add(bl[:], u[:], bl[:])
            sg = small.tile([128, CP], F32, tag=f"sg{s}")
            nc.vector.tensor_scalar(out=sg[:], in0=bl[:], scalar1=0.5,
                                    scalar2=0.5, op0=ALU.mult, op1=ALU.add)
            return sg

        def scale_tile(s, ci, sg):
            t = xt[(s, ci)]
            nc.vector.tensor_scalar_mul(t[:], t[:], sg[:, ci:ci + 1])

        def store_tile(s, ci, split=False):
            t = xt[(s, ci)]
            eng = nc.sync if K_ST_SYNC else nc.scalar
            cuts = [0, HL, L] if split else [0, L]
            for a, b in zip(cuts[:-1], cuts[1:]):
                eng.dma_start(out=o_d[s, ci * 128:(ci + 1) * 128, a:b],
                              in_=t[:, a:b])

        # ---- sample 0: reduce, gate, then scale/store interleaved with
        # ---- sample 1's reductions (deadline order on V and A) -----------
        red_sample(0)
        merge(0)
        tk0, thp0 = gate_mm(0)
        red_v(1, 0)                      # fill V while gate0's PE/A work
        red_a(1, 0, HL, L, 1)
        sg0 = gate_blend(0, tk0, thp0)

        scale_tile(0, 0, sg0)
        store_tile(0, 0)
        red_v(1, 1)
        red_a(1, 1, HL, L, 1)
        scale_tile(0, 1, sg0)
        store_tile(0, 1)
        red_v(1, 2)
        red_a(1, 2, HL, L, 1)
        scale_tile(0, 2, sg0)
        store_tile(0, 2)
        red_v(1, 3)
        red_a(1, 3, HL, HL + QL, 1)
        red_a(1, 3, HL + QL, L, 2)
        scale_tile(0, 3, sg0)
        store_tile(0, 3)

        merge(1)
        tk1, thp1 = gate_mm(1)
        sg1 = gate_blend(1, tk1, thp1)
        for ci in range(CP):
            scale_tile(1, ci, sg1)
            store_tile(1, ci, split=(ci == CP - 1))

    nc.compile()
    return nc


_COMPILED = {}


def _get_compiled(b2_val):
    key = float(b2_val)
    if key not in _COMPILED:
        _COMPILED[key] = _build(key)
    return _COMPILED[key]


def _make_consts(w1, b1, w2, b2):
    w1 = np.asarray(w1, np.float32)
    b1 = np.asarray(b1, np.float32)
    w2 = np.asarray(w2, np.float32)

    S17 = np.zeros((128, 17, 128), np.float32)
    p = np.arange(128)
    for j in range(9):
        d = j - 4
        m = (p + d >= 0) & (p + d < 128)
        S17[p[m] + d, j, p[m]] = 1.0
    for d in range(1, 5):
        m = p + d - 128 >= 0
        S17[p[m] + d - 128, 8 + d, p[m]] = 1.0
    for d in range(-4, 0):
        m = p + d + 128 < 128
        S17[p[m] + d + 128, 17 + d, p[m]] = 1.0

    cstb = np.zeros((128, CB_COLS), np.float32)
    # w1t: [CP, 128, HID] flattened as CP blocks of HID columns, 1/L folded
    w1t = (w1.T / np.float32(L)).reshape(CP, 128, HID)
    for i in range(CP):
        cstb[:, W1T_OFF + i * HID:W1T_OFF + (i + 1) * HID] = w1t[i]
    j9 = np.arange(9)
    for kidx, k in enumerate((3, 5, 7)):
        w = ((np.abs(j9 - 4) <= (k - 1) // 2).astype(np.float32)
             / np.float32(k) / np.float32(L))
        bands = [sum(w[j] * S17[:, j, :] for j in range(9)),
                 sum(w[d + 4] * S17[:, 8 + d, :] for d in range(1, 5)),
                 sum(w[d + 4] * S17[:, 17 + d, :] for d in range(-4, 0))]
        for m in range(3):
            o = WB_OFF + (m * 3 + kidx) * 128
            cstb[:, o:o + 128] = bands[m]
    cstf = np.zeros((HID, 2), np.float32)
    cstf[:, 0] = b1
    cstf[:, 1] = w2.reshape(HID)
    return {"cstb": np.ascontiguousarray(cstb.astype(ml_dtypes.bfloat16)),
            "cstf": np.ascontiguousarray(cstf)}


def kernel(x, w1, b1, w2, b2):
    x = np.asarray(x, np.float32)
    assert x.shape == (B, C, L), x.shape
    nc = _get_compiled(np.float32(np.asarray(b2).reshape(-1)[0]))
    consts = _make_consts(w1, b1, w2, b2)
    xb = np.ascontiguousarray(x).astype(ml_dtypes.bfloat16)
    in_maps = []
    for i in range(N_CORES):
        m = {"x": np.ascontiguousarray(xb[i * BS:(i + 1) * BS])}
        m.update(consts)
        in_maps.append(m)
    res = run_bass_kernel_spmd(nc, in_maps, list(range(N_CORES)),
                               trace=bool(int(os.environ.get("K_TRACE", "0"))))
    out = np.concatenate(
        [np.asarray(res.results[i]["out"]).astype(np.float32)
         for i in range(N_CORES)], axis=0)
    if res.exec_time_ns is not None:
        kernel.last_exec_time_ns = res.exec_time_ns
        kernel.last_mean_exec_time_ns = res.mean_exec_time_ns
    kernel.last_results = res
    return out
